# revision 38
# baseline (speedup 1.0000x reference)
"""Trainium2 Bass kernel for nn_AttentionPruneKV (sparse attention with
dual-RPE bias, dynamic per-query prune threshold, and attn0 side output).

Sharding: one head per NeuronCore (8 heads / 8 cores), each core handles all
4 batches for its head.  Math notes:

  attn = e/s with e = exp(D - m), D = dots0 + C, C = ctab[rel_index]
  (ctab = t1*t2 + 0.01*exp(-factor*dis-table); rel_index/dis are the canonical
  2D-relative grids, so C is block-Toeplitz and is expanded on-device with a
  strided DMA read of the 3969-entry table).
  record  = attn > thresh  <=>  e > tau,  tau = emin + r*(1 - emin) in e-space
  out_row = (sum_j U_j v_j) / (E + 1e-6*s),  U = e*record, E = sum_j U_j
  so attn never needs materializing and deno comes free from the custom DVE
  select op's accumulator.  The 1/(E+1e-6*s) row scale commutes through the
  W_out projection (per-head), so it is applied on the PV result during PSUM
  evacuation.
"""

import math
import operator

import numpy as np

HEADS = 8
DIM_HEAD = 64
H = W = 32
N = H * W  # 1024
B = 4
DIM = 512
TABLE = (2 * H - 1) * (2 * W - 1)  # 3969
NEG_THRESH = 0.9
GATE = -2.0
NCORES = 8

_CACHE = {}


def _canon_tables():
    """Canonical rel_index plus the dis-values-by-table-index vector."""
    coords = np.stack(np.meshgrid(np.arange(H), np.arange(W), indexing="ij"))
    flat = coords.reshape(2, -1)
    rel = flat[:, :, None] - flat[:, None, :]
    dis = ((rel[0] / H) ** 2 + (rel[1] / W) ** 2).astype(np.float32)
    idx = ((rel[0] + H - 1) * (2 * W - 1) + (rel[1] + W - 1)).astype(np.int32)
    dr = np.arange(TABLE) // (2 * W - 1) - (H - 1)
    dc = np.arange(TABLE) % (2 * W - 1) - (W - 1)
    g = ((dr / H) ** 2 + (dc / W) ** 2).astype(np.float32)
    return idx, dis, g


def _register_dve_op(name, spec, rd1_en):
    from concourse import dve_ops
    from concourse.dve_spec import lower
    from concourse.dve_uop import DveOpSpec

    for op in dve_ops.OPS:
        if op.name == name:
            return op
    row = dve_ops._CUSTOM_DVE_ROW_BASE + len(dve_ops.OPS)
    shas = {}
    for ver in ("v3", "v4"):
        s = DveOpSpec(name=name, opcode=row, uops=lower(spec, ver=ver),
                      rd1_en=rd1_en)
        shas[ver] = s.sha(ver)
    op = dve_ops.DveOp(name, spec, subdim=False, uops_sha=shas)
    dve_ops.OPS.append(op)
    dve_ops.CUSTOM_DVE_SPECS[name] = spec
    dve_ops._SUB_OPCODE_FOR_NAME[name] = row
    return op


def _register_select_op():
    """Custom DVE op: out = (in0 > s0) ? in0 : 0 ; accum_out = sum(out)."""
    from concourse.dve_spec import Spec, Src0, C0, Zero, select

    def _ref(in0, in1, s0, s1, imm2):
        b = np.where(in0.astype(np.float32) > s0, in0, 0.0).astype(np.float32)
        return b, b.reshape(b.shape[0], -1).sum(axis=-1, keepdims=True)

    spec = Spec(
        body=select(Src0 > C0, Src0, Zero),
        accum=operator.add,
        accum_init=Zero,
        reference=_ref,
    )
    return _register_dve_op("SELECT_GT_SUM_ANT_X", spec, rd1_en=False)


def _register_addmax_op():
    """Custom DVE op: out = in0 + in1 ; accum_out = max(s0, rowmax(out)).

    (The stock TENSOR_TENSOR_REDUCE instruction faults on TRN2 hardware, so
    the fused add+rowmax runs as a custom DVE program instead.)"""
    from concourse.dve_spec import Spec, Src0, Src1, C0, maxx

    def _ref(in0, in1, s0, s1, imm2):
        b = (in0.astype(np.float32) + in1).astype(np.float32)
        return b, np.maximum(
            s0, b.reshape(b.shape[0], -1).max(axis=-1, keepdims=True)
        )

    spec = Spec(body=Src0 + Src1, accum=maxx, accum_init=C0, reference=_ref)
    return _register_dve_op("TT_ADD_MAX_ANT_X", spec, rd1_en=True)


def _build_program(n_pairs=B, with_pv=True, steps=99):
    import concourse.bass as bass
    import concourse.mybir as mybir
    import concourse.tile as tile
    from concourse import bacc
    from concourse.masks import make_identity
    from contextlib import ExitStack

    f32 = mybir.dt.float32
    Alu = mybir.AluOpType
    Act = mybir.ActivationFunctionType
    X = mybir.AxisListType.X

    selop = _register_select_op()
    addmax = _register_addmax_op()

    nc = bacc.Bacc(target_bir_lowering=False, debug=False)
    xt_d = nc.dram_tensor("xt", [DIM, B * N], f32, kind="ExternalInput")
    wa_d = nc.dram_tensor("wa", [128, 4, 128], f32, kind="ExternalInput")
    wb_d = nc.dram_tensor("wb", [128, 4, 64], f32, kind="ExternalInput")
    wo_d = nc.dram_tensor("wo", [64, 512], f32, kind="ExternalInput")
    cf_d = nc.dram_tensor("cf", [128, 8, 1024], f32, kind="ExternalInput")
    rr_d = nc.dram_tensor("rr", [128, 32], f32, kind="ExternalInput")
    a0_d = nc.dram_tensor("a0", [B, N, N], f32, kind="ExternalOutput")
    pj_d = nc.dram_tensor("pj", [B * N, 512], f32, kind="ExternalOutput")

    with ExitStack() as ctx:
        tc = ctx.enter_context(tile.TileContext(nc))
        persist = ctx.enter_context(tc.tile_pool(name="persist", bufs=1))
        xpool = ctx.enter_context(tc.tile_pool(name="xpool", bufs=2))
        work = ctx.enter_context(tc.tile_pool(name="work", bufs=2))
        outp = ctx.enter_context(tc.tile_pool(name="outp", bufs=2))
        stats = ctx.enter_context(tc.tile_pool(name="stats", bufs=4))
        wpool = ctx.enter_context(tc.tile_pool(name="wpool", bufs=2))
        ppool = ctx.enter_context(tc.tile_pool(name="ppool", bufs=2, space="PSUM"))
        pdots = ctx.enter_context(tc.tile_pool(name="pdots", bufs=2, space="PSUM"))
        ptr = ctx.enter_context(tc.tile_pool(name="ptr", bufs=2, space="PSUM"))

        ident = persist.tile([128, 128], f32)
        make_identity(nc, ident)
        # weight tiles consumed as matmul operands go through an ACT-copy hop
        # so matmuls never carry more than one DMA-queue semaphore wait
        wa_s = persist.tile([128, 4, 128], f32)
        nc.sync.dma_start(out=wa_s, in_=wa_d[:, :, :])
        wa = persist.tile([128, 4, 128], f32)
        nc.scalar.copy(wa, wa_s)
        wb_s = persist.tile([128, 4, 64], f32)
        nc.sync.dma_start(out=wb_s, in_=wb_d[:, :, :])
        wb = persist.tile([128, 4, 64], f32)
        nc.scalar.copy(wb, wb_s)
        f32r = mybir.dt.float32r
        wo_s = persist.tile([64, 512], f32)
        nc.sync.dma_start(out=wo_s, in_=wo_d[:, :])
        wo = persist.tile([64, 512], f32r)
        nc.scalar.copy(wo, wo_s)
        rr = persist.tile([128, 32], f32)
        nc.sync.dma_start(out=rr, in_=rr_d[:, :])

        # C[i%128, i-tile t, j] = bias+pos table expanded on host; one DMA
        C = persist.tile([128, 8, 1024], f32)
        nc.sync.dma_start(out=C, in_=cf_d[:, :, :])

        # qkv: QV rows 0:64 = q_hat (prescaled by 1/8), rows 64:128 = v_hat
        QV = persist.tile([128, B * N], f32)
        K = persist.tile([64, B * N], f32)
        for sl in range(8):
            xsl = xpool.tile([128, 4, 512], f32, tag="xsl")
            src = bass.AP(
                tensor=xt_d[:, :].tensor,
                offset=sl * 512,
                ap=[[B * N, 128], [128 * B * N, 4], [1, 512]],
            )
            nc.sync.dma_start(out=xsl, in_=src)
            psa = ppool.tile([128, 512], f32, tag="mm")
            for kt in range(4):
                nc.tensor.matmul(
                    psa, lhsT=wa[:, kt, :], rhs=xsl[:, kt, :],
                    start=(kt == 0), stop=(kt == 3),
                )
            nc.scalar.copy(QV[:, sl * 512 : (sl + 1) * 512], psa)
            psb = ppool.tile([128, 512], f32, tag="mm")
            for kt in range(4):
                nc.tensor.matmul(
                    psb[0:64, :], lhsT=wb[:, kt, :], rhs=xsl[:, kt, :],
                    start=(kt == 0), stop=(kt == 3),
                )
            nc.scalar.copy(K[:, sl * 512 : (sl + 1) * 512], psb[0:64, :])

        # v_hat (rows 64:128 of QV) -> V natural (j%128, chunk g, d), rounded
        # to f32r for the single-pass PV matmul
        V = persist.tile([128, 32, 64], f32r)
        for gq in range(8):
            pst = ptr.tile([128, 512], f32, tag="tr")
            for u in range(4):
                g = gq * 4 + u
                nc.tensor.transpose(
                    pst[:, u * 64 : (u + 1) * 64],
                    QV[64:128, g * 128 : (g + 1) * 128],
                    ident[64:128, 64:128],
                )
            nc.scalar.copy(V[:, gq * 4 : (gq + 1) * 4, :], pst[:, 0:256])

        UT = persist.tile([128, 8, 1024], f32r)  # [j%128, j-chunk, i]
        for b in range(n_pairs):
            w_pair = wpool.tile([128, 8], f32, tag="w")
            for t in range(8):
                pd = pdots.tile([128, 1024], f32, tag="dots")
                qs = QV[0:64, b * N + t * 128 : b * N + (t + 1) * 128]
                nc.tensor.matmul(
                    pd[:, 0:512], lhsT=qs, rhs=K[:, b * N : b * N + 512],
                    start=True, stop=True,
                )
                nc.tensor.matmul(
                    pd[:, 512:1024], lhsT=qs, rhs=K[:, b * N + 512 : b * N + 1024],
                    start=True, stop=True,
                )
                if steps < 2:
                    continue
                D = work.tile([128, 1024], f32, tag="D")
                rmax = stats.tile([128, 1], f32, tag="rmax")
                nc.vector._custom_dve(
                    addmax, out=D, in0=pd, in1=C[:, t, :], s0=-3.0e38,
                    accum_out=rmax,
                )
                rmin = stats.tile([128, 1], f32, tag="rmin")
                nc.vector.tensor_reduce(out=rmin, in_=D, axis=X, op=Alu.min)
                if steps < 3:
                    continue
                negm = stats.tile([128, 1], f32, tag="negm")
                nc.scalar.mul(negm, rmax, -1.0)
                # attn0 = exp(dots0 - m) / sum(exp(dots0 - m)); shift cancels
                a0t = outp.tile([128, 1024], f32, tag="a0")
                s0 = stats.tile([128, 1], f32, tag="s0")
                nc.scalar.activation(a0t, pd, Act.Exp, bias=negm, scale=1.0,
                                     accum_out=s0)
                e = work.tile([128, 1024], f32, tag="e")
                s = stats.tile([128, 1], f32, tag="s")
                nc.scalar.activation(e, D, Act.Exp, bias=negm, scale=1.0,
                                     accum_out=s)
                emin = stats.tile([128, 1], f32, tag="emin")
                nc.scalar.activation(emin, rmin, Act.Exp, bias=negm, scale=1.0)
                if steps < 4:
                    continue
                rs0 = stats.tile([128, 1], f32, tag="rs0")
                nc.vector.reciprocal(rs0, s0)
                nc.gpsimd.tensor_scalar_mul(a0t, a0t, rs0)
                nc.sync.dma_start(out=a0_d[b, t * 128 : (t + 1) * 128, :], in_=a0t)
                if steps < 5:
                    continue
                # tau = emin + r*(1 - emin)   (r preloaded; 1e30 on gated rows)
                tmp = stats.tile([128, 1], f32, tag="tmp")
                nc.vector.tensor_scalar(tmp, emin, -1.0, 1.0,
                                        op0=Alu.mult, op1=Alu.add)
                tau = stats.tile([128, 1], f32, tag="tau")
                nc.vector.scalar_tensor_tensor(
                    out=tau, in0=rr[:, b * 8 + t : b * 8 + t + 1], scalar=tmp,
                    op0=Alu.mult, op1=Alu.add, in1=emin,
                )
                U = work.tile([128, 1024], f32, tag="U")
                E = stats.tile([128, 1], f32, tag="E")
                nc.vector._custom_dve(selop, out=U, in0=e, s0=tau, accum_out=E)
                den = stats.tile([128, 1], f32, tag="den")
                nc.vector.scalar_tensor_tensor(
                    out=den, in0=s, scalar=1e-6, op0=Alu.mult, op1=Alu.add, in1=E,
                )
                nc.vector.reciprocal(w_pair[:, t : t + 1], den)
                if steps < 6:
                    continue
                for jh in range(2):
                    pu = ptr.tile([128, 512], f32, tag="tr")
                    for u in range(4):
                        jc = jh * 4 + u
                        nc.tensor.transpose(
                            pu[:, u * 128 : (u + 1) * 128],
                            U[:, jc * 128 : (jc + 1) * 128],
                            ident,
                        )
                    nc.scalar.copy(
                        UT[:, jh * 4 : (jh + 1) * 4, t * 128 : (t + 1) * 128],
                        pu[:].rearrange("p (a c) -> p a c", a=4),
                    )
            if not with_pv:
                continue
            # PV (fp32r single-pass): oT[d, i] = sum_j V[j,d] * UT[j,i]
            po = pdots.tile([128, 1024], f32, tag="dots")
            for jc in range(8):
                for hh in range(2):
                    nc.tensor.matmul(
                        po[0:64, hh * 512 : (hh + 1) * 512],
                        lhsT=V[:, b * 8 + jc, :],
                        rhs=UT[:, jc, hh * 512 : (hh + 1) * 512],
                        start=(jc == 0), stop=(jc == 7),
                    )
            oT = wpool.tile([64, 1024], f32r, tag="oT")
            nc.scalar.copy(oT, po[0:64, :])
            # proj (fp32r); the 1/(E + 1e-6 s) row scale rides the PSUM copy
            for t in range(8):
                pp = ppool.tile([128, 512], f32, tag="mm")
                nc.tensor.matmul(
                    pp, lhsT=oT[:, t * 128 : (t + 1) * 128], rhs=wo,
                    start=True, stop=True,
                )
                pjt = outp.tile([128, 512], f32, tag="pj")
                nc.scalar.mul(pjt, pp, w_pair[:, t : t + 1])
                nc.sync.dma_start(
                    out=pj_d[b * N + t * 128 : b * N + (t + 1) * 128, :], in_=pjt
                )
    nc.finalize()
    return nc


def _get_program():
    if "nc" not in _CACHE:
        _CACHE["nc"] = _build_program()
    return _CACHE["nc"]


def _idxmap():
    """idxF[p, t, j] = table index for C[i=128t+p, j] (block-Toeplitz)."""
    if "idxmap" not in _CACHE:
        p = np.arange(128)
        t = np.arange(8)
        rj = np.arange(N) // 32
        cj = np.arange(N) % 32
        ri = 4 * t[None, :, None] + (p[:, None, None] // 32)
        ci = p[:, None, None] % 32
        idx = (1984 + 63 * ri + ci) - 63 * rj[None, None, :] - cj[None, None, :]
        _CACHE["idxmap"] = idx.astype(np.int32)
    return _CACHE["idxmap"]


def _sigmoid32(x):
    x = x.astype(np.float32)
    return np.float32(1.0) / (np.float32(1.0) + np.exp(-x, dtype=np.float32))


def _prep_inmaps(x, prob, W_qkv, table1, table2, headsita, W_thresh, W_out,
                 b_out, rel_index, dis):
    x = np.asarray(x, np.float32)
    prob = np.asarray(prob, np.float32)
    W_qkv = np.asarray(W_qkv, np.float32)
    table1 = np.asarray(table1, np.float32)
    table2 = np.asarray(table2, np.float32)
    headsita = np.asarray(headsita, np.float32)
    W_thresh = np.asarray(W_thresh, np.float32)
    W_out = np.asarray(W_out, np.float32)
    b_out = np.asarray(b_out, np.float32)
    rel_index = np.asarray(rel_index)
    dis = np.asarray(dis, np.float32)

    idx_c, dis_c, g = _canon_tables()
    if not (np.array_equal(rel_index, idx_c) and np.array_equal(dis, dis_c)):
        raise NotImplementedError(
            "kernel compiled for the canonical rel_index/dis grids"
        )

    inner = HEADS * DIM_HEAD
    xf = np.ascontiguousarray(x.reshape(B * N, DIM))
    xt = np.ascontiguousarray(xf.T)  # (512, 4096)

    factor = (np.float32(1.0) /
              (np.float32(2.0) * headsita * headsita + np.float32(1e-6)))
    sig_gate = _sigmoid32(np.float32(GATE))

    in_maps = []
    for h in range(NCORES):
        wq = W_qkv[:, h * 64 : (h + 1) * 64]
        wk = W_qkv[:, inner + h * 64 : inner + (h + 1) * 64]
        wv = W_qkv[:, 2 * inner + h * 64 : 2 * inner + (h + 1) * 64]
        wa_cat = np.concatenate([wq * np.float32(0.125), wv], axis=1)  # (512,128)
        wa_l = np.ascontiguousarray(wa_cat.reshape(4, 128, 128).transpose(1, 0, 2))
        wb_l = np.ascontiguousarray(wk.reshape(4, 128, 64).transpose(1, 0, 2))
        wo_l = np.ascontiguousarray(W_out[h * 64 : (h + 1) * 64, :])

        pos_t = np.exp(-factor[h] * g, dtype=np.float32)
        ct = table1[:, h] * table2[:, h] + np.float32(0.01) * pos_t
        cf = ct[_idxmap()]  # (128, 8, 1024) block-Toeplitz expansion

        q_h = xf @ wq  # (4096, 64) fp32, matches reference order
        logits = q_h @ W_thresh  # (4096, 1)
        r = _sigmoid32(logits[:, 0]) * sig_gate
        r = np.where(prob.reshape(B * N) >= np.float32(NEG_THRESH),
                     np.float32(1e30), r).astype(np.float32)
        rr = np.ascontiguousarray(
            r.reshape(B, 8, 128).transpose(2, 0, 1).reshape(128, 32)
        )

        in_maps.append({
            "xt": xt, "wa": wa_l, "wb": wb_l, "wo": wo_l, "cf": cf, "rr": rr,
        })
    return in_maps


def kernel(x, prob, W_qkv, table1, table2, headsita, W_thresh, W_out, b_out,
           rel_index, dis):
    b_out = np.asarray(b_out, np.float32)
    in_maps = _prep_inmaps(x, prob, W_qkv, table1, table2, headsita, W_thresh,
                           W_out, b_out, rel_index, dis)

    nc = _get_program()
    from concourse.bass_utils import run_bass_kernel_spmd

    import os
    import time as _time
    trace = bool(os.environ.get("KERNEL_TRACE"))
    _t0 = _time.monotonic()
    res = run_bass_kernel_spmd(
        nc, in_maps, core_ids=list(range(NCORES)), trace=trace
    )
    _CACHE["spmd_wall_ns"] = int((_time.monotonic() - _t0) * 1e9)
    results = res.results
    _CACHE["last_result"] = res

    attn0 = np.empty((B, HEADS, N, N), np.float32)
    out = np.zeros((B * N, 512), np.float32)
    for h in range(NCORES):
        attn0[:, h] = results[h]["a0"]
        out += results[h]["pj"]
    out = out.reshape(B, N, 512) + b_out
    return out, attn0


if __name__ == "__main__":
    rng = np.random.default_rng(0)
    idx_c, dis_c, g = _canon_tables()
    inputs = dict(
        x=rng.standard_normal((B, N, DIM)).astype(np.float32),
        prob=rng.random((B, N)).astype(np.float32),
        W_qkv=(rng.standard_normal((DIM, 3 * HEADS * DIM_HEAD)) * 0.02).astype(np.float32),
        table1=(rng.standard_normal((TABLE, HEADS)) * 0.02).astype(np.float32),
        table2=(rng.standard_normal((TABLE, HEADS)) * 0.02).astype(np.float32),
        headsita=(np.arange(1, HEADS + 1) * 0.1).astype(np.float32),
        W_thresh=(rng.standard_normal((DIM_HEAD, 1)) * 0.02).astype(np.float32),
        W_out=(rng.standard_normal((HEADS * DIM_HEAD, 512)) * 0.02).astype(np.float32),
        b_out=np.zeros(512, np.float32),
        rel_index=idx_c,
        dis=dis_c,
    )
    out, attn0 = kernel(**inputs)
    print("out", out.shape, out.dtype, "attn0", attn0.shape)


# revision 40
# speedup vs baseline: 1.0081x; 1.0081x over previous
"""Trainium2 Bass kernel for nn_AttentionPruneKV (sparse attention with
dual-RPE bias, dynamic per-query prune threshold, and attn0 side output).

Sharding: one head per NeuronCore (8 heads / 8 cores), each core handles all
4 batches for its head.  Math notes:

  attn = e/s with e = exp(D - m), D = dots0 + C, C = ctab[rel_index]
  (ctab = t1*t2 + 0.01*exp(-factor*dis-table); rel_index/dis are the canonical
  2D-relative grids, so C is block-Toeplitz and is expanded on-device with a
  strided DMA read of the 3969-entry table).
  record  = attn > thresh  <=>  e > tau,  tau = emin + r*(1 - emin) in e-space
  out_row = (sum_j U_j v_j) / (E + 1e-6*s),  U = e*record, E = sum_j U_j
  so attn never needs materializing and deno comes free from the custom DVE
  select op's accumulator.  The 1/(E+1e-6*s) row scale commutes through the
  W_out projection (per-head), so it is applied on the PV result during PSUM
  evacuation.
"""

import math
import operator

import numpy as np

HEADS = 8
DIM_HEAD = 64
H = W = 32
N = H * W  # 1024
B = 4
DIM = 512
TABLE = (2 * H - 1) * (2 * W - 1)  # 3969
NEG_THRESH = 0.9
GATE = -2.0
NCORES = 8

_CACHE = {}


def _canon_tables():
    """Canonical rel_index plus the dis-values-by-table-index vector."""
    coords = np.stack(np.meshgrid(np.arange(H), np.arange(W), indexing="ij"))
    flat = coords.reshape(2, -1)
    rel = flat[:, :, None] - flat[:, None, :]
    dis = ((rel[0] / H) ** 2 + (rel[1] / W) ** 2).astype(np.float32)
    idx = ((rel[0] + H - 1) * (2 * W - 1) + (rel[1] + W - 1)).astype(np.int32)
    dr = np.arange(TABLE) // (2 * W - 1) - (H - 1)
    dc = np.arange(TABLE) % (2 * W - 1) - (W - 1)
    g = ((dr / H) ** 2 + (dc / W) ** 2).astype(np.float32)
    return idx, dis, g


def _register_dve_op(name, spec, rd1_en):
    from concourse import dve_ops
    from concourse.dve_spec import lower
    from concourse.dve_uop import DveOpSpec

    for op in dve_ops.OPS:
        if op.name == name:
            return op
    row = dve_ops._CUSTOM_DVE_ROW_BASE + len(dve_ops.OPS)
    shas = {}
    for ver in ("v3", "v4"):
        s = DveOpSpec(name=name, opcode=row, uops=lower(spec, ver=ver),
                      rd1_en=rd1_en)
        shas[ver] = s.sha(ver)
    op = dve_ops.DveOp(name, spec, subdim=False, uops_sha=shas)
    dve_ops.OPS.append(op)
    dve_ops.CUSTOM_DVE_SPECS[name] = spec
    dve_ops._SUB_OPCODE_FOR_NAME[name] = row
    return op


def _register_select_op():
    """Custom DVE op: out = (in0 > s0) ? in0 : 0 ; accum_out = sum(out)."""
    from concourse.dve_spec import Spec, Src0, C0, Zero, select

    def _ref(in0, in1, s0, s1, imm2):
        b = np.where(in0.astype(np.float32) > s0, in0, 0.0).astype(np.float32)
        return b, b.reshape(b.shape[0], -1).sum(axis=-1, keepdims=True)

    spec = Spec(
        body=select(Src0 > C0, Src0, Zero),
        accum=operator.add,
        accum_init=Zero,
        reference=_ref,
    )
    return _register_dve_op("SELECT_GT_SUM_ANT_X", spec, rd1_en=False)


def _register_addmax_op():
    """Custom DVE op: out = in0 + in1 ; accum_out = max(s0, rowmax(out)).

    (The stock TENSOR_TENSOR_REDUCE instruction faults on TRN2 hardware, so
    the fused add+rowmax runs as a custom DVE program instead.)"""
    from concourse.dve_spec import Spec, Src0, Src1, C0, maxx

    def _ref(in0, in1, s0, s1, imm2):
        b = (in0.astype(np.float32) + in1).astype(np.float32)
        return b, np.maximum(
            s0, b.reshape(b.shape[0], -1).max(axis=-1, keepdims=True)
        )

    spec = Spec(body=Src0 + Src1, accum=maxx, accum_init=C0, reference=_ref)
    return _register_dve_op("TT_ADD_MAX_ANT_X", spec, rd1_en=True)


def _build_program(n_pairs=B, with_pv=True, steps=99):
    import concourse.bass as bass
    import concourse.mybir as mybir
    import concourse.tile as tile
    from concourse import bacc
    from concourse.masks import make_identity
    from contextlib import ExitStack

    f32 = mybir.dt.float32
    Alu = mybir.AluOpType
    Act = mybir.ActivationFunctionType
    X = mybir.AxisListType.X

    selop = _register_select_op()
    addmax = _register_addmax_op()

    nc = bacc.Bacc(target_bir_lowering=False, debug=False)
    xt_d = nc.dram_tensor("xt", [DIM, B * N], f32, kind="ExternalInput")
    wa_d = nc.dram_tensor("wa", [128, 4, 128], f32, kind="ExternalInput")
    wb_d = nc.dram_tensor("wb", [128, 4, 64], f32, kind="ExternalInput")
    wo_d = nc.dram_tensor("wo", [64, 512], f32, kind="ExternalInput")
    cf_d = nc.dram_tensor("cf", [128, 8, 1024], f32, kind="ExternalInput")
    rr_d = nc.dram_tensor("rr", [128, 32], f32, kind="ExternalInput")
    a0_d = nc.dram_tensor("a0", [B, N, N], f32, kind="ExternalOutput")
    pj_d = nc.dram_tensor("pj", [B * N, 512], f32, kind="ExternalOutput")

    with ExitStack() as ctx:
        tc = ctx.enter_context(tile.TileContext(nc))
        persist = ctx.enter_context(tc.tile_pool(name="persist", bufs=1))
        xpool = ctx.enter_context(tc.tile_pool(name="xpool", bufs=2))
        work = ctx.enter_context(tc.tile_pool(name="work", bufs=2))
        outp = ctx.enter_context(tc.tile_pool(name="outp", bufs=2))
        stats = ctx.enter_context(tc.tile_pool(name="stats", bufs=4))
        wpool = ctx.enter_context(tc.tile_pool(name="wpool", bufs=2))
        ppool = ctx.enter_context(tc.tile_pool(name="ppool", bufs=2, space="PSUM"))
        pdots = ctx.enter_context(tc.tile_pool(name="pdots", bufs=2, space="PSUM"))
        ptr = ctx.enter_context(tc.tile_pool(name="ptr", bufs=2, space="PSUM"))

        ident = persist.tile([128, 128], f32)
        make_identity(nc, ident)
        # weight tiles consumed as matmul operands go through an ACT-copy hop
        # so matmuls never carry more than one DMA-queue semaphore wait
        wa_s = persist.tile([128, 4, 128], f32)
        nc.sync.dma_start(out=wa_s, in_=wa_d[:, :, :])
        wa = persist.tile([128, 4, 128], f32)
        nc.scalar.copy(wa, wa_s)
        wb_s = persist.tile([128, 4, 64], f32)
        nc.sync.dma_start(out=wb_s, in_=wb_d[:, :, :])
        wb = persist.tile([128, 4, 64], f32)
        nc.scalar.copy(wb, wb_s)
        f32r = mybir.dt.float32r
        wo_s = persist.tile([64, 512], f32)
        nc.sync.dma_start(out=wo_s, in_=wo_d[:, :])
        wo = persist.tile([64, 512], f32r)
        nc.scalar.copy(wo, wo_s)
        rr = persist.tile([128, 32], f32)
        nc.sync.dma_start(out=rr, in_=rr_d[:, :])

        # C[i%128, i-tile t, j] = bias+pos table expanded on host; one DMA
        C = persist.tile([128, 8, 1024], f32)
        nc.sync.dma_start(out=C, in_=cf_d[:, :, :])

        # qkv: QV rows 0:64 = q_hat (prescaled by 1/8), rows 64:128 = v_hat
        QV = persist.tile([128, B * N], f32)
        K = persist.tile([64, B * N], f32)
        for sl in range(8):
            xsl = xpool.tile([128, 4, 512], f32, tag="xsl")
            src = bass.AP(
                tensor=xt_d[:, :].tensor,
                offset=sl * 512,
                ap=[[B * N, 128], [128 * B * N, 4], [1, 512]],
            )
            nc.sync.dma_start(out=xsl, in_=src)
            psa = ppool.tile([128, 512], f32, tag="mm")
            for kt in range(4):
                nc.tensor.matmul(
                    psa, lhsT=wa[:, kt, :], rhs=xsl[:, kt, :],
                    start=(kt == 0), stop=(kt == 3),
                )
            nc.scalar.copy(QV[:, sl * 512 : (sl + 1) * 512], psa)
            psb = ppool.tile([128, 512], f32, tag="mm")
            for kt in range(4):
                nc.tensor.matmul(
                    psb[0:64, :], lhsT=wb[:, kt, :], rhs=xsl[:, kt, :],
                    start=(kt == 0), stop=(kt == 3),
                )
            nc.scalar.copy(K[:, sl * 512 : (sl + 1) * 512], psb[0:64, :])

        # v_hat (rows 64:128 of QV) -> V natural (j%128, chunk g, d), rounded
        # to f32r for the single-pass PV matmul
        V = persist.tile([128, 32, 64], f32r)
        for gq in range(8):
            pst = ptr.tile([128, 512], f32, tag="tr")
            for u in range(4):
                g = gq * 4 + u
                nc.tensor.transpose(
                    pst[:, u * 64 : (u + 1) * 64],
                    QV[64:128, g * 128 : (g + 1) * 128],
                    ident[64:128, 64:128],
                )
            nc.scalar.copy(V[:, gq * 4 : (gq + 1) * 4, :], pst[:, 0:256])

        UT = persist.tile([128, 8, 1024], f32r)  # [j%128, j-chunk, i]
        for b in range(n_pairs):
            w_pair = wpool.tile([128, 8], f32, tag="w")
            for t in range(8):
                pd = pdots.tile([128, 1024], f32, tag="dots")
                qs = QV[0:64, b * N + t * 128 : b * N + (t + 1) * 128]
                nc.tensor.matmul(
                    pd[:, 0:512], lhsT=qs, rhs=K[:, b * N : b * N + 512],
                    start=True, stop=True,
                )
                nc.tensor.matmul(
                    pd[:, 512:1024], lhsT=qs, rhs=K[:, b * N + 512 : b * N + 1024],
                    start=True, stop=True,
                )
                if steps < 2:
                    continue
                D = work.tile([128, 1024], f32, tag="D")
                rmax = stats.tile([128, 1], f32, tag="rmax")
                nc.vector._custom_dve(
                    addmax, out=D, in0=pd, in1=C[:, t, :], s0=-3.0e38,
                    accum_out=rmax,
                )
                rmin = stats.tile([128, 1], f32, tag="rmin")
                nc.vector.tensor_reduce(out=rmin, in_=D, axis=X, op=Alu.min)
                if steps < 3:
                    continue
                negm = stats.tile([128, 1], f32, tag="negm")
                nc.scalar.mul(negm, rmax, -1.0)
                # attn0 = exp(dots0 - m) / sum(exp(dots0 - m)); shift cancels
                a0t = outp.tile([128, 1024], f32, tag="a0")
                s0 = stats.tile([128, 1], f32, tag="s0")
                nc.scalar.activation(a0t, pd, Act.Exp, bias=negm, scale=1.0,
                                     accum_out=s0)
                e = work.tile([128, 1024], f32, tag="e")
                s = stats.tile([128, 1], f32, tag="s")
                nc.scalar.activation(e, D, Act.Exp, bias=negm, scale=1.0,
                                     accum_out=s)
                emin = stats.tile([128, 1], f32, tag="emin")
                nc.scalar.activation(emin, rmin, Act.Exp, bias=negm, scale=1.0)
                if steps < 4:
                    continue
                rs0 = stats.tile([128, 1], f32, tag="rs0")
                nc.vector.reciprocal(rs0, s0)
                nc.gpsimd.tensor_scalar_mul(a0t, a0t, rs0)
                nc.sync.dma_start(out=a0_d[b, t * 128 : (t + 1) * 128, :], in_=a0t)
                if steps < 5:
                    continue
                # tau = emin + r*(1 - emin)   (r preloaded; 1e30 on gated rows)
                tmp = stats.tile([128, 1], f32, tag="tmp")
                nc.vector.tensor_scalar(tmp, emin, -1.0, 1.0,
                                        op0=Alu.mult, op1=Alu.add)
                tau = stats.tile([128, 1], f32, tag="tau")
                nc.vector.scalar_tensor_tensor(
                    out=tau, in0=rr[:, b * 8 + t : b * 8 + t + 1], scalar=tmp,
                    op0=Alu.mult, op1=Alu.add, in1=emin,
                )
                U = work.tile([128, 1024], f32, tag="U")
                E = stats.tile([128, 1], f32, tag="E")
                nc.vector._custom_dve(selop, out=U, in0=e, s0=tau, accum_out=E)
                den = stats.tile([128, 1], f32, tag="den")
                nc.vector.scalar_tensor_tensor(
                    out=den, in0=s, scalar=1e-6, op0=Alu.mult, op1=Alu.add, in1=E,
                )
                nc.vector.reciprocal(w_pair[:, t : t + 1], den)
                if steps < 6:
                    continue
                for jh in range(2):
                    pu = ptr.tile([128, 512], f32, tag="tr")
                    for u in range(4):
                        jc = jh * 4 + u
                        nc.tensor.transpose(
                            pu[:, u * 128 : (u + 1) * 128],
                            U[:, jc * 128 : (jc + 1) * 128],
                            ident,
                        )
                    nc.scalar.copy(
                        UT[:, jh * 4 : (jh + 1) * 4, t * 128 : (t + 1) * 128],
                        pu[:].rearrange("p (a c) -> p a c", a=4),
                    )
            if not with_pv:
                continue
            # PV (fp32r single-pass): oT[d, i] = sum_j V[j,d] * UT[j,i]
            po = pdots.tile([128, 1024], f32, tag="dots")
            for jc in range(8):
                for hh in range(2):
                    nc.tensor.matmul(
                        po[0:64, hh * 512 : (hh + 1) * 512],
                        lhsT=V[:, b * 8 + jc, :],
                        rhs=UT[:, jc, hh * 512 : (hh + 1) * 512],
                        start=(jc == 0), stop=(jc == 7),
                    )
            oT = wpool.tile([64, 1024], f32r, tag="oT")
            nc.scalar.copy(oT, po[0:64, :])
            # proj (fp32r); the 1/(E + 1e-6 s) row scale rides the PSUM copy
            for t in range(8):
                pp = ppool.tile([128, 512], f32, tag="mm")
                nc.tensor.matmul(
                    pp, lhsT=oT[:, t * 128 : (t + 1) * 128], rhs=wo,
                    start=True, stop=True,
                )
                pjt = outp.tile([128, 512], f32, tag="pj")
                nc.scalar.mul(pjt, pp, w_pair[:, t : t + 1])
                nc.sync.dma_start(
                    out=pj_d[b * N + t * 128 : b * N + (t + 1) * 128, :], in_=pjt
                )
    nc.finalize()
    return nc


def _get_program():
    if "nc" not in _CACHE:
        _CACHE["nc"] = _build_program()
    return _CACHE["nc"]


def _idxmap():
    """idxF[p, t, j] = table index for C[i=128t+p, j] (block-Toeplitz)."""
    if "idxmap" not in _CACHE:
        p = np.arange(128)
        t = np.arange(8)
        rj = np.arange(N) // 32
        cj = np.arange(N) % 32
        ri = 4 * t[None, :, None] + (p[:, None, None] // 32)
        ci = p[:, None, None] % 32
        idx = (1984 + 63 * ri + ci) - 63 * rj[None, None, :] - cj[None, None, :]
        _CACHE["idxmap"] = idx.astype(np.int32)
    return _CACHE["idxmap"]


def _sigmoid32(x):
    x = x.astype(np.float32)
    return np.float32(1.0) / (np.float32(1.0) + np.exp(-x, dtype=np.float32))


def _prep_inmaps(x, prob, W_qkv, table1, table2, headsita, W_thresh, W_out,
                 b_out, rel_index, dis):
    x = np.asarray(x, np.float32)
    prob = np.asarray(prob, np.float32)
    W_qkv = np.asarray(W_qkv, np.float32)
    table1 = np.asarray(table1, np.float32)
    table2 = np.asarray(table2, np.float32)
    headsita = np.asarray(headsita, np.float32)
    W_thresh = np.asarray(W_thresh, np.float32)
    W_out = np.asarray(W_out, np.float32)
    b_out = np.asarray(b_out, np.float32)
    rel_index = np.asarray(rel_index)
    dis = np.asarray(dis, np.float32)

    idx_c, dis_c, g = _canon_tables()
    canonical = np.array_equal(rel_index, idx_c) and np.array_equal(dis, dis_c)

    inner = HEADS * DIM_HEAD
    xf = np.ascontiguousarray(x.reshape(B * N, DIM))
    xt = np.ascontiguousarray(xf.T)  # (512, 4096)

    factor = (np.float32(1.0) /
              (np.float32(2.0) * headsita * headsita + np.float32(1e-6)))
    sig_gate = _sigmoid32(np.float32(GATE))

    in_maps = []
    for h in range(NCORES):
        wq = W_qkv[:, h * 64 : (h + 1) * 64]
        wk = W_qkv[:, inner + h * 64 : inner + (h + 1) * 64]
        wv = W_qkv[:, 2 * inner + h * 64 : 2 * inner + (h + 1) * 64]
        wa_cat = np.concatenate([wq * np.float32(0.125), wv], axis=1)  # (512,128)
        wa_l = np.ascontiguousarray(wa_cat.reshape(4, 128, 128).transpose(1, 0, 2))
        wb_l = np.ascontiguousarray(wk.reshape(4, 128, 64).transpose(1, 0, 2))
        wo_l = np.ascontiguousarray(W_out[h * 64 : (h + 1) * 64, :])

        if canonical:
            pos_t = np.exp(-factor[h] * g, dtype=np.float32)
            ct = table1[:, h] * table2[:, h] + np.float32(0.01) * pos_t
            cf = ct[_idxmap()]  # (128, 8, 1024) block-Toeplitz expansion
        else:
            prod = table1[:, h] * table2[:, h]
            pos = np.exp(-factor[h] * dis, dtype=np.float32)
            cfull = prod[rel_index] + np.float32(0.01) * pos  # (1024, 1024)
            cf = np.ascontiguousarray(
                cfull.reshape(8, 128, N).transpose(1, 0, 2)
            )

        q_h = xf @ wq  # (4096, 64) fp32, matches reference order
        logits = q_h @ W_thresh  # (4096, 1)
        r = _sigmoid32(logits[:, 0]) * sig_gate
        r = np.where(prob.reshape(B * N) >= np.float32(NEG_THRESH),
                     np.float32(1e30), r).astype(np.float32)
        rr = np.ascontiguousarray(
            r.reshape(B, 8, 128).transpose(2, 0, 1).reshape(128, 32)
        )

        in_maps.append({
            "xt": xt, "wa": wa_l, "wb": wb_l, "wo": wo_l, "cf": cf, "rr": rr,
        })
    return in_maps


def kernel(x, prob, W_qkv, table1, table2, headsita, W_thresh, W_out, b_out,
           rel_index, dis):
    b_out = np.asarray(b_out, np.float32)
    in_maps = _prep_inmaps(x, prob, W_qkv, table1, table2, headsita, W_thresh,
                           W_out, b_out, rel_index, dis)

    nc = _get_program()
    from concourse.bass_utils import run_bass_kernel_spmd

    import os
    import time as _time
    trace = bool(os.environ.get("KERNEL_TRACE"))
    _t0 = _time.monotonic()
    res = run_bass_kernel_spmd(
        nc, in_maps, core_ids=list(range(NCORES)), trace=trace
    )
    _CACHE["spmd_wall_ns"] = int((_time.monotonic() - _t0) * 1e9)
    results = res.results
    _CACHE["last_result"] = res

    attn0 = np.empty((B, HEADS, N, N), np.float32)
    out = np.zeros((B * N, 512), np.float32)
    for h in range(NCORES):
        attn0[:, h] = results[h]["a0"]
        out += results[h]["pj"]
    out = out.reshape(B, N, 512) + b_out
    return out, attn0


if __name__ == "__main__":
    rng = np.random.default_rng(0)
    idx_c, dis_c, g = _canon_tables()
    inputs = dict(
        x=rng.standard_normal((B, N, DIM)).astype(np.float32),
        prob=rng.random((B, N)).astype(np.float32),
        W_qkv=(rng.standard_normal((DIM, 3 * HEADS * DIM_HEAD)) * 0.02).astype(np.float32),
        table1=(rng.standard_normal((TABLE, HEADS)) * 0.02).astype(np.float32),
        table2=(rng.standard_normal((TABLE, HEADS)) * 0.02).astype(np.float32),
        headsita=(np.arange(1, HEADS + 1) * 0.1).astype(np.float32),
        W_thresh=(rng.standard_normal((DIM_HEAD, 1)) * 0.02).astype(np.float32),
        W_out=(rng.standard_normal((HEADS * DIM_HEAD, 512)) * 0.02).astype(np.float32),
        b_out=np.zeros(512, np.float32),
        rel_index=idx_c,
        dis=dis_c,
    )
    out, attn0 = kernel(**inputs)
    print("out", out.shape, out.dtype, "attn0", attn0.shape)
